# revision 5
# baseline (speedup 1.0000x reference)
"""Trainium2 Bass kernel for nn_Conv2d_uint8_custom (dynamic uint8 quant + LUT conv).

Semantics (matches reference.py):
  qf = clip(round(x/scale_f) + zero_f, 0, 255)          (per-tensor dynamic quant)
  qw = clip(round(w/scale_w) + zero_w, 0, 255)
  acc[b,o,l] = sum_k lut[qf_patch, qw] = sum_k qf*qw     (lut is an exact product table)
  out = (acc - zero_f * qw_sum[o]) * scale_f * scale_w + bias[o]

Strategy (v2):
  * batch-parallel across 8 cores (2 images per core)
  * ALL quantization on host (exact fp32 replication of the reference);
    device receives pre-quantized fp16 features (ints 0..255, exact in fp16)
    already laid out in the padded [58x58] geometry with the row-shifted
    partition halves pre-packed -> the device is a pure GEMM + epilogue
  * 3x3 conv: per 448-px output tile, 6 matmuls: (kh=0,kh=1) tap pairs packed
    to K=128 via the pre-shifted feature half; kh=2 rides K=64 with zeroed
    weight halves
  * PE warmup: dummy matmuls during the load phase ramp the tensor engine
    p-state to 2.4GHz before the first real matmul
  * epilogue scale+bias in fp16 output; host converts back to fp32
"""

import numpy as np
import ml_dtypes
from contextlib import ExitStack

import concourse.bass as bass
import concourse.tile as tile
from concourse import bacc, mybir


def _ensure_axon_ntff_hook():
    """This image's `antenv` lacks `axon_hooks`, which bass_utils imports
    unconditionally when tracing under axon. Provide it (backed by the ctypes
    NTFF hook from trn_agent_boot when available, else None so concourse
    degrades to an untraced run)."""
    import sys, types

    if "antenv.axon_hooks" in sys.modules:
        return
    try:
        import antenv
    except ImportError:
        return
    mod = types.ModuleType("antenv.axon_hooks")
    hook = [None]
    try:
        from trn_agent_boot.trn_boot import _ntff_profile_via_ctypes

        hook[0] = _ntff_profile_via_ctypes("/opt/axon/libaxon_pjrt.so")
    except Exception:
        pass
    mod.get_axon_ntff_profile_hook = lambda: hook[0]
    mod.set_axon_ntff_profile_hook = lambda h: hook.__setitem__(0, h)
    sys.modules["antenv.axon_hooks"] = mod
    antenv.axon_hooks = mod


_ensure_axon_ntff_hook()

N_CORES = 8
B, C, H, W = 16, 64, 56, 56
O = 128
IMG_PER_CORE = B // N_CORES  # 2
L = H * W                    # 3136
HP, WP = H + 2, W + 2        # 58, 58 (zero-padded layout)
LP = HP * WP                 # 3364
TILE_ROWS = 8
NT = H // TILE_ROWS          # 7 output tiles per image
NCOL = TILE_ROWS * W         # 448 columns per tile (one PSUM bank)
N_WARM = 8                   # PE p-state warmup matmuls (big)
N_WARM_SMALL = 10            # trailing fine-grained warmups
WARM_COLS = 256

FP32 = mybir.dt.float32
BF16 = mybir.dt.bfloat16

# feature-plane load chunks (padded-row ranges); first small so tile 0's
# data (rows 0..9) lands as early as possible
CHUNKS = [(0, 10), (10, 26), (26, 42), (42, 58)]

_NC = None


def _build_nc():
    nc = bacc.Bacc(
        "TRN2",
        debug=False,
        enable_asserts=False,
        num_devices=N_CORES,
        enable_partition_id=False,
    )
    fq_d = nc.dram_tensor("fq", [4, 128, LP], BF16, kind="ExternalInput").ap()
    wq_d = nc.dram_tensor("wq", [2, 128, 5, 128], BF16, kind="ExternalInput").ap()
    qp_d = nc.dram_tensor("qp", [128, 2], FP32, kind="ExternalInput").ap()
    out_d = nc.dram_tensor(
        "out", [IMG_PER_CORE, O, L], BF16, kind="ExternalOutput"
    ).ap()

    with tile.TileContext(nc) as tc:
        with ExitStack() as ctx:
            _body(ctx, tc, fq_d, wq_d, qp_d, out_d)
    nc.compile()
    return nc


def _body(ctx, tc, fq_d, wq_d, qp_d, out_d):
    nc = tc.nc
    A = mybir.AluOpType
    ID = mybir.ActivationFunctionType.Identity
    consts = ctx.enter_context(tc.tile_pool(name="consts", bufs=1))
    fpool = ctx.enter_context(tc.tile_pool(name="feat", bufs=1))
    opool = ctx.enter_context(tc.tile_pool(name="osb", bufs=4))
    ppool = ctx.enter_context(tc.tile_pool(name="acc", bufs=7, space="PSUM"))
    wpool = ctx.enter_context(tc.tile_pool(name="warm", bufs=1, space="PSUM"))

    # warmup fodder: a zero tile the dummy matmuls read (dep: memset only)
    warm = consts.tile([128, WARM_COLS], BF16)
    nc.gpsimd.memset(warm[:], 0.0)

    # weights [img, K, g, O]: img0/g0 slice first on the sync ring so the
    # first LDWEIGHTS' dependency lands as early as possible
    wq = consts.tile([128, 2, 5, 128], BF16)
    qp = consts.tile([128, 2], FP32)

    F0 = fpool.tile([128, LP], BF16, name="F0")
    F1 = fpool.tile([128, LP], BF16, name="F1")
    M0 = fpool.tile([128, LP], BF16, name="M0")
    M1 = fpool.tile([128, LP], BF16, name="M1")

    # sync ring: first-needed things in latency order
    nc.sync.dma_start(wq[:, 0, 0:1], wq_d[0, :, 0:1])
    a, b = CHUNKS[0]
    nc.sync.dma_start(M0[:, a * WP : b * WP], fq_d[2, :, a * WP : b * WP])
    nc.sync.dma_start(F0[:, a * WP : b * WP], fq_d[0, :, a * WP : b * WP])
    nc.sync.dma_start(wq[:, 0, 1:5], wq_d[0, :, 1:5])
    for a, b in CHUNKS[1:]:
        nc.sync.dma_start(F0[:, a * WP : b * WP], fq_d[0, :, a * WP : b * WP])
    # scalar ring: M0 rest (needed early for img0 g=0..2), then F1
    for a, b in CHUNKS[1:]:
        nc.scalar.dma_start(M0[:, a * WP : b * WP], fq_d[2, :, a * WP : b * WP])
    for a, b in CHUNKS:
        nc.scalar.dma_start(F1[:, a * WP : b * WP], fq_d[1, :, a * WP : b * WP])
    # pool ring: epilogue consts first (first epilogue ~t+4.5us), img1
    # weights, then the img1 M plane (needed last)
    nc.gpsimd.dma_start(qp[:], qp_d[:])
    nc.gpsimd.dma_start(wq[:, 1], wq_d[1])
    for a, b in CHUNKS:
        nc.gpsimd.dma_start(M1[:, a * WP : b * WP], fq_d[3, :, a * WP : b * WP])

    F0v = F0[:].rearrange("p (r c) -> p r c", c=WP)
    F1v = F1[:].rearrange("p (r c) -> p r c", c=WP)
    M0v = M0[:].rearrange("p (r c) -> p r c", c=WP)
    M1v = M1[:].rearrange("p (r c) -> p r c", c=WP)

    # PE p-state warmup: dummy matmuls on the zero tile keep the tensor
    # engine continuously busy through the DMA/semaphore latency of the
    # first chunks (big ones ramp, small trailing ones avoid an idle gap
    # that would drop the p-state before the first real matmul).
    for k in range(N_WARM + N_WARM_SMALL):
        cols = WARM_COLS if k < N_WARM else 32
        pw = wpool.tile([128, cols], FP32, name=f"pw{k}", tag="pw")
        nc.tensor.matmul(
            pw[:], warm[:, 0:128], warm[:, 0:cols],
            start=True, stop=True, skip_group_check=True,
        )

    # GEMM: per image, 7 tiles of [128 oc, 448 px]; per tile 5 matmuls:
    # g=0..2: M plane rows rt+g  -> taps (kh=g, kw=0)+(kh=g, kw=1)  K=128
    # g=3:    F plane rows rt+0, col 2 -> (kh0,kw2)+(kh1,kw2)       K=128
    # g=4:    F plane rows rt+2, col 2 -> (kh2,kw2)                 K=64
    for img in range(IMG_PER_CORE):
        fv = F0v if img == 0 else F1v
        mv = M0v if img == 0 else M1v
        for t in range(NT):
            ps = ppool.tile([128, NCOL], FP32, name=f"ps{img}_{t}", tag="ps")
            for g in range(5):
                if g < 3:
                    src_v = mv[:, TILE_ROWS * t + g : TILE_ROWS * t + g + TILE_ROWS, 0:W]
                else:
                    rt = TILE_ROWS * t + (0 if g == 3 else 2)
                    src_v = fv[:, rt : rt + TILE_ROWS, 2 : 2 + W]
                nc.tensor.matmul(
                    ps[:],
                    wq[:, img, g, :],
                    src_v,
                    start=(g == 0),
                    stop=(g == 4),
                    skip_group_check=True,
                )
            # epilogue: out = psum * s_tot + bias_eff, bf16, paired tiles;
            # the very last tile splits across both engines to shorten the tail
            half = t % 2
            if half == 0:
                width = NCOL * (2 if t + 1 < NT else 1)
                o_sb = opool.tile([128, 2 * NCOL], BF16, name="o_sb")
            dst = o_sb[:, half * NCOL : (half + 1) * NCOL]
            if img == IMG_PER_CORE - 1 and t == NT - 1:
                hw_ = NCOL // 2
                nc.scalar.activation(
                    dst[:, 0:hw_], ps[:, 0:hw_], ID, bias=qp[:, 0:1], scale=qp[:, 1:2]
                )
                nc.vector.tensor_scalar(
                    dst[:, hw_:NCOL], ps[:, hw_:NCOL], qp[:, 1:2], qp[:, 0:1],
                    op0=A.mult, op1=A.add,
                )
            elif (img * NT + t) % 2 == 0:
                nc.scalar.activation(
                    dst, ps[:], ID, bias=qp[:, 0:1], scale=qp[:, 1:2]
                )
            else:
                nc.vector.tensor_scalar(
                    dst, ps[:], qp[:, 1:2], qp[:, 0:1], op0=A.mult, op1=A.add
                )
            if half == 1 or t == NT - 1:
                c0 = (t - half) * NCOL
                nc.sync.dma_start(
                    out_d[img, :, c0 : c0 + width], o_sb[:, 0:width]
                )


def _prep_host(x, weight, bias):
    """Exact fp32 replication of the reference's quantization arithmetic
    (numpy and jax-on-cpu are both IEEE fp32, round-half-even), then pack
    the padded/shifted fp16 feature planes, fp16 weights, and the folded
    epilogue scale/bias."""
    f = np.float32
    mx, mn = f(x.max()), f(x.min())
    scale_f = f((mx - mn) / f(255.0))
    zero_f = f(-np.round(mn / scale_f))
    qf = np.clip(
        np.round(x.astype(np.float32) / scale_f) + zero_f, 0.0, 255.0
    ).astype(ml_dtypes.bfloat16)  # exact small ints

    mw, nw = f(weight.max()), f(weight.min())
    scale_w = f((mw - nw) / f(255.0))
    zero_w = f(-np.round(nw / scale_w))
    qw = np.clip(
        np.round(weight.astype(np.float32) / scale_w) + zero_w, 0.0, 255.0
    ).astype(np.float32)  # exact small ints

    s_tot = f(scale_f * scale_w)
    qw_sum = qw.reshape(O, -1).sum(axis=1, dtype=np.float64)
    bias_eff = (
        bias.astype(np.float64) - np.float64(zero_f) * qw_sum * np.float64(s_tot)
    ).astype(np.float32)
    qp = np.zeros((128, 2), np.float32)
    qp[:, 0] = bias_eff
    qp[:, 1] = s_tot

    # padded features [B, C, 58*58] + shifted variants:
    # up1 (flat +58) for the kh pairing, left1 (flat +1) for the kw pairing
    pad = np.zeros((B, C, HP, WP), ml_dtypes.bfloat16)
    pad[:, :, 1 : 1 + H, 1 : 1 + W] = qf
    flat = pad.reshape(B, C, LP)
    shU = np.zeros_like(flat)
    shU[:, :, : LP - WP] = flat[:, :, WP:]
    shL = np.zeros_like(flat)
    shL[:, :, : LP - 1] = flat[:, :, 1:]

    # per-core planes [4, 128, LP]: F0, F1, M0, M1
    fq_cores = []
    for c in range(N_CORES):
        i0, i1 = 2 * c, 2 * c + 1
        p_f0 = np.concatenate([flat[i0], shU[i0]], axis=0)  # img0 | img0-up1
        p_f1 = np.concatenate([shU[i1], flat[i1]], axis=0)  # img1-up1 | img1
        p_m0 = np.concatenate([flat[i0], shL[i0]], axis=0)  # img0 | img0-left1
        p_m1 = np.concatenate([shL[i1], flat[i1]], axis=0)  # img1-left1 | img1
        fq_cores.append(
            np.ascontiguousarray(np.stack([p_f0, p_f1, p_m0, p_m1]))
        )

    # weights [img, 128 (K), 5 (g), 128 (O)]
    qwT = qw.transpose(2, 3, 1, 0)  # [kh, kw, C, O]
    wqa = np.zeros((2, 128, 5, 128), np.float32)
    for g in range(3):  # M-plane groups: (kh=g, kw0) + (kh=g, kw1)
        wqa[0, 0:64, g] = qwT[g, 0]
        wqa[0, 64:128, g] = qwT[g, 1]
        wqa[1, 0:64, g] = qwT[g, 1]    # img1 lo = left-shifted -> kw1
        wqa[1, 64:128, g] = qwT[g, 0]
    # g=3: F plane rows rt, col 2 -> (kh0,kw2) lo + (kh1,kw2) hi (img0)
    wqa[0, 0:64, 3] = qwT[0, 2]
    wqa[0, 64:128, 3] = qwT[1, 2]
    wqa[1, 0:64, 3] = qwT[1, 2]       # img1 lo = up-shifted -> kh1
    wqa[1, 64:128, 3] = qwT[0, 2]
    # g=4: F plane rows rt+2, col 2 -> (kh2,kw2), K=64
    wqa[0, 0:64, 4] = qwT[2, 2]
    wqa[1, 64:128, 4] = qwT[2, 2]
    return fq_cores, wqa.astype(ml_dtypes.bfloat16), qp


def build():
    global _NC
    if _NC is None:
        _NC = _build_nc()
    return _NC


LAST_RESULT = None


def kernel(x, weight, bias, lut):
    global LAST_RESULT
    from concourse.bass_utils import run_bass_kernel_spmd

    x = np.asarray(x, dtype=np.float32)
    weight = np.asarray(weight, dtype=np.float32)
    bias = np.asarray(bias, dtype=np.float32)

    fq_cores, wq, qp = _prep_host(x, weight, bias)
    nc = build()
    in_maps = [
        {"fq": fq_cores[c], "wq": wq, "qp": qp} for c in range(N_CORES)
    ]

    res = run_bass_kernel_spmd(nc, in_maps, core_ids=list(range(N_CORES)))
    LAST_RESULT = res
    out = np.concatenate(
        [r["out"].reshape(IMG_PER_CORE, O, H, W) for r in res.results], axis=0
    )
    return out.astype(np.float32)


# revision 6
# speedup vs baseline: 1.0324x; 1.0324x over previous
"""Trainium2 Bass kernel for nn_Conv2d_uint8_custom (dynamic uint8 quant + LUT conv).

Semantics (matches reference.py):
  qf = clip(round(x/scale_f) + zero_f, 0, 255)          (per-tensor dynamic quant)
  qw = clip(round(w/scale_w) + zero_w, 0, 255)
  acc[b,o,l] = sum_k lut[qf_patch, qw] = sum_k qf*qw     (lut is an exact product table)
  out = (acc - zero_f * qw_sum[o]) * scale_f * scale_w + bias[o]

Strategy (v2):
  * batch-parallel across 8 cores (2 images per core)
  * ALL quantization on host (exact fp32 replication of the reference);
    device receives pre-quantized fp16 features (ints 0..255, exact in fp16)
    already laid out in the padded [58x58] geometry with the row-shifted
    partition halves pre-packed -> the device is a pure GEMM + epilogue
  * 3x3 conv: per 448-px output tile, 6 matmuls: (kh=0,kh=1) tap pairs packed
    to K=128 via the pre-shifted feature half; kh=2 rides K=64 with zeroed
    weight halves
  * PE warmup: dummy matmuls during the load phase ramp the tensor engine
    p-state to 2.4GHz before the first real matmul
  * epilogue scale+bias in fp16 output; host converts back to fp32
"""

import numpy as np
import ml_dtypes
from contextlib import ExitStack

import concourse.bass as bass
import concourse.tile as tile
from concourse import bacc, mybir


def _ensure_axon_ntff_hook():
    """This image's `antenv` lacks `axon_hooks`, which bass_utils imports
    unconditionally when tracing under axon. Provide it (backed by the ctypes
    NTFF hook from trn_agent_boot when available, else None so concourse
    degrades to an untraced run)."""
    import sys, types

    if "antenv.axon_hooks" in sys.modules:
        return
    try:
        import antenv
    except ImportError:
        return
    mod = types.ModuleType("antenv.axon_hooks")
    hook = [None]
    try:
        from trn_agent_boot.trn_boot import _ntff_profile_via_ctypes

        hook[0] = _ntff_profile_via_ctypes("/opt/axon/libaxon_pjrt.so")
    except Exception:
        pass
    mod.get_axon_ntff_profile_hook = lambda: hook[0]
    mod.set_axon_ntff_profile_hook = lambda h: hook.__setitem__(0, h)
    sys.modules["antenv.axon_hooks"] = mod
    antenv.axon_hooks = mod


_ensure_axon_ntff_hook()

N_CORES = 8
B, C, H, W = 16, 64, 56, 56
O = 128
IMG_PER_CORE = B // N_CORES  # 2
L = H * W                    # 3136
HP, WP = H + 2, W + 2        # 58, 58 (zero-padded layout)
LP = HP * WP                 # 3364
TILE_ROWS = 8
NT = H // TILE_ROWS          # 7 output tiles per image
NCOL = TILE_ROWS * W         # 448 columns per tile (one PSUM bank)
N_WARM = 7                   # PE p-state warmup matmuls (big)
N_WARM_SMALL = 12            # trailing fine-grained warmups
WARM_COLS = 256

FP32 = mybir.dt.float32
BF16 = mybir.dt.bfloat16

# feature-plane load chunks (padded-row ranges); first small so tile 0's
# data (rows 0..9) lands as early as possible
CHUNKS = [(0, 10), (10, 26), (26, 42), (42, 58)]

_NC = None


def _build_nc():
    nc = bacc.Bacc(
        "TRN2",
        debug=False,
        enable_asserts=False,
        num_devices=N_CORES,
        enable_partition_id=False,
    )
    fq_d = nc.dram_tensor("fq", [4, 128, LP], BF16, kind="ExternalInput").ap()
    wq_d = nc.dram_tensor("wq", [2, 128, 5, 128], BF16, kind="ExternalInput").ap()
    qp_d = nc.dram_tensor("qp", [128, 2], FP32, kind="ExternalInput").ap()
    out_d = nc.dram_tensor(
        "out", [IMG_PER_CORE, O, L], BF16, kind="ExternalOutput"
    ).ap()

    with tile.TileContext(nc) as tc:
        with ExitStack() as ctx:
            _body(ctx, tc, fq_d, wq_d, qp_d, out_d)
    nc.compile()
    return nc


def _body(ctx, tc, fq_d, wq_d, qp_d, out_d):
    nc = tc.nc
    A = mybir.AluOpType
    ID = mybir.ActivationFunctionType.Identity
    consts = ctx.enter_context(tc.tile_pool(name="consts", bufs=1))
    fpool = ctx.enter_context(tc.tile_pool(name="feat", bufs=1))
    opool = ctx.enter_context(tc.tile_pool(name="osb", bufs=4))
    ppool = ctx.enter_context(tc.tile_pool(name="acc", bufs=7, space="PSUM"))
    wpool = ctx.enter_context(tc.tile_pool(name="warm", bufs=1, space="PSUM"))

    warm = consts.tile([128, WARM_COLS], BF16)
    wq = consts.tile([128, 2, 5, 128], BF16)
    qp = consts.tile([128, 2], FP32)

    F0 = fpool.tile([128, LP], BF16, name="F0")
    F1 = fpool.tile([128, LP], BF16, name="F1")
    M0 = fpool.tile([128, LP], BF16, name="M0")
    M1 = fpool.tile([128, LP], BF16, name="M1")

    # pool ring exits its preamble earliest: first-LDW weights, epilogue
    # consts, img1 weights, then the img1 M plane (needed last)
    nc.gpsimd.dma_start(wq[:, 0, 0:1], wq_d[0, :, 0:1])
    # warmup fodder: a zero tile the dummy matmuls read (dep: memset only)
    nc.gpsimd.memset(warm[:], 0.0)
    nc.gpsimd.dma_start(qp[:], qp_d[:])
    nc.gpsimd.dma_start(wq[:, 1], wq_d[1])
    for a, b in CHUNKS:
        nc.gpsimd.dma_start(M1[:, a * WP : b * WP], fq_d[3, :, a * WP : b * WP])
    # sync ring: M plane chunk 0 (feeds the very first matmul), remaining
    # img0 weights, rest of F0
    a, b = CHUNKS[0]
    nc.sync.dma_start(M0[:, a * WP : b * WP], fq_d[2, :, a * WP : b * WP])
    nc.sync.dma_start(wq[:, 0, 1:5], wq_d[0, :, 1:5])
    for a, b in CHUNKS[1:]:
        nc.sync.dma_start(F0[:, a * WP : b * WP], fq_d[0, :, a * WP : b * WP])
    # scalar ring: F0 chunk 0 (4th matmul), M0 rest, then F1
    a, b = CHUNKS[0]
    nc.scalar.dma_start(F0[:, a * WP : b * WP], fq_d[0, :, a * WP : b * WP])
    for a, b in CHUNKS[1:]:
        nc.scalar.dma_start(M0[:, a * WP : b * WP], fq_d[2, :, a * WP : b * WP])
    for a, b in CHUNKS:
        nc.scalar.dma_start(F1[:, a * WP : b * WP], fq_d[1, :, a * WP : b * WP])

    F0v = F0[:].rearrange("p (r c) -> p r c", c=WP)
    F1v = F1[:].rearrange("p (r c) -> p r c", c=WP)
    M0v = M0[:].rearrange("p (r c) -> p r c", c=WP)
    M1v = M1[:].rearrange("p (r c) -> p r c", c=WP)

    # PE p-state warmup: dummy matmuls on the zero tile keep the tensor
    # engine continuously busy through the DMA/semaphore latency of the
    # first chunks (big ones ramp, small trailing ones avoid an idle gap
    # that would drop the p-state before the first real matmul).
    pw = wpool.tile([128, WARM_COLS], FP32, name="pw", tag="pw")
    n_all = N_WARM + N_WARM_SMALL
    for k in range(n_all):
        cols = WARM_COLS if k < N_WARM else 32
        nc.tensor.matmul(
            pw[:, 0:cols], warm[:, 0:128], warm[:, 0:cols],
            start=(k == 0), stop=(k == n_all - 1), skip_group_check=True,
        )

    # GEMM: per image, 7 tiles of [128 oc, 448 px]; per tile 5 matmuls:
    # g=0..2: M plane rows rt+g  -> taps (kh=g, kw=0)+(kh=g, kw=1)  K=128
    # g=3:    F plane rows rt+0, col 2 -> (kh0,kw2)+(kh1,kw2)       K=128
    # g=4:    F plane rows rt+2, col 2 -> (kh2,kw2)                 K=64
    for img in range(IMG_PER_CORE):
        fv = F0v if img == 0 else F1v
        mv = M0v if img == 0 else M1v
        for t in range(NT):
            ps = ppool.tile([128, NCOL], FP32, name=f"ps{img}_{t}", tag="ps")
            for g in range(5):
                if g < 3:
                    src_v = mv[:, TILE_ROWS * t + g : TILE_ROWS * t + g + TILE_ROWS, 0:W]
                else:
                    rt = TILE_ROWS * t + (0 if g == 3 else 2)
                    src_v = fv[:, rt : rt + TILE_ROWS, 2 : 2 + W]
                nc.tensor.matmul(
                    ps[:],
                    wq[:, img, g, :],
                    src_v,
                    start=(g == 0),
                    stop=(g == 4),
                    skip_group_check=True,
                )
            # epilogue: out = psum * s_tot + bias_eff, bf16, paired tiles;
            # the very last tile splits across both engines to shorten the tail
            half = t % 2
            if half == 0:
                width = NCOL * (2 if t + 1 < NT else 1)
                o_sb = opool.tile([128, 2 * NCOL], BF16, name="o_sb")
            dst = o_sb[:, half * NCOL : (half + 1) * NCOL]
            if (img * NT + t) % 2 == 0:
                nc.scalar.activation(
                    dst, ps[:], ID, bias=qp[:, 0:1], scale=qp[:, 1:2]
                )
            else:
                nc.vector.tensor_scalar(
                    dst, ps[:], qp[:, 1:2], qp[:, 0:1], op0=A.mult, op1=A.add
                )
            if half == 1 or t == NT - 1:
                c0 = (t - half) * NCOL
                nc.sync.dma_start(
                    out_d[img, :, c0 : c0 + width], o_sb[:, 0:width]
                )


def _prep_host(x, weight, bias):
    """Exact fp32 replication of the reference's quantization arithmetic
    (numpy and jax-on-cpu are both IEEE fp32, round-half-even), then pack
    the padded/shifted fp16 feature planes, fp16 weights, and the folded
    epilogue scale/bias."""
    f = np.float32
    mx, mn = f(x.max()), f(x.min())
    scale_f = f((mx - mn) / f(255.0))
    zero_f = f(-np.round(mn / scale_f))
    qf = np.clip(
        np.round(x.astype(np.float32) / scale_f) + zero_f, 0.0, 255.0
    ).astype(ml_dtypes.bfloat16)  # exact small ints

    mw, nw = f(weight.max()), f(weight.min())
    scale_w = f((mw - nw) / f(255.0))
    zero_w = f(-np.round(nw / scale_w))
    qw = np.clip(
        np.round(weight.astype(np.float32) / scale_w) + zero_w, 0.0, 255.0
    ).astype(np.float32)  # exact small ints

    s_tot = f(scale_f * scale_w)
    qw_sum = qw.reshape(O, -1).sum(axis=1, dtype=np.float64)
    bias_eff = (
        bias.astype(np.float64) - np.float64(zero_f) * qw_sum * np.float64(s_tot)
    ).astype(np.float32)
    qp = np.zeros((128, 2), np.float32)
    qp[:, 0] = bias_eff
    qp[:, 1] = s_tot

    # padded features [B, C, 58*58] + shifted variants:
    # up1 (flat +58) for the kh pairing, left1 (flat +1) for the kw pairing
    pad = np.zeros((B, C, HP, WP), ml_dtypes.bfloat16)
    pad[:, :, 1 : 1 + H, 1 : 1 + W] = qf
    flat = pad.reshape(B, C, LP)
    shU = np.zeros_like(flat)
    shU[:, :, : LP - WP] = flat[:, :, WP:]
    shL = np.zeros_like(flat)
    shL[:, :, : LP - 1] = flat[:, :, 1:]

    # per-core planes [4, 128, LP]: F0, F1, M0, M1
    fq_cores = []
    for c in range(N_CORES):
        i0, i1 = 2 * c, 2 * c + 1
        p_f0 = np.concatenate([flat[i0], shU[i0]], axis=0)  # img0 | img0-up1
        p_f1 = np.concatenate([shU[i1], flat[i1]], axis=0)  # img1-up1 | img1
        p_m0 = np.concatenate([flat[i0], shL[i0]], axis=0)  # img0 | img0-left1
        p_m1 = np.concatenate([shL[i1], flat[i1]], axis=0)  # img1-left1 | img1
        fq_cores.append(
            np.ascontiguousarray(np.stack([p_f0, p_f1, p_m0, p_m1]))
        )

    # weights [img, 128 (K), 5 (g), 128 (O)]
    qwT = qw.transpose(2, 3, 1, 0)  # [kh, kw, C, O]
    wqa = np.zeros((2, 128, 5, 128), np.float32)
    for g in range(3):  # M-plane groups: (kh=g, kw0) + (kh=g, kw1)
        wqa[0, 0:64, g] = qwT[g, 0]
        wqa[0, 64:128, g] = qwT[g, 1]
        wqa[1, 0:64, g] = qwT[g, 1]    # img1 lo = left-shifted -> kw1
        wqa[1, 64:128, g] = qwT[g, 0]
    # g=3: F plane rows rt, col 2 -> (kh0,kw2) lo + (kh1,kw2) hi (img0)
    wqa[0, 0:64, 3] = qwT[0, 2]
    wqa[0, 64:128, 3] = qwT[1, 2]
    wqa[1, 0:64, 3] = qwT[1, 2]       # img1 lo = up-shifted -> kh1
    wqa[1, 64:128, 3] = qwT[0, 2]
    # g=4: F plane rows rt+2, col 2 -> (kh2,kw2), K=64
    wqa[0, 0:64, 4] = qwT[2, 2]
    wqa[1, 64:128, 4] = qwT[2, 2]
    return fq_cores, wqa.astype(ml_dtypes.bfloat16), qp


def build():
    global _NC
    if _NC is None:
        _NC = _build_nc()
    return _NC


LAST_RESULT = None


def kernel(x, weight, bias, lut):
    global LAST_RESULT
    from concourse.bass_utils import run_bass_kernel_spmd

    x = np.asarray(x, dtype=np.float32)
    weight = np.asarray(weight, dtype=np.float32)
    bias = np.asarray(bias, dtype=np.float32)

    fq_cores, wq, qp = _prep_host(x, weight, bias)
    nc = build()
    in_maps = [
        {"fq": fq_cores[c], "wq": wq, "qp": qp} for c in range(N_CORES)
    ]

    res = run_bass_kernel_spmd(nc, in_maps, core_ids=list(range(N_CORES)))
    LAST_RESULT = res
    out = np.concatenate(
        [r["out"].reshape(IMG_PER_CORE, O, H, W) for r in res.results], axis=0
    )
    return out.astype(np.float32)
